# revision 3
# baseline (speedup 1.0000x reference)
"""Trainium2 Bass kernel: MultiHeadAttention (B=4, S=2048, D=1024, H=16).

Sharding: 8 cores, each handles (batch b = core//2, query half = core%2):
  - projects q for its 1024 query rows, k/v for the full 2048-row sequence
    of its batch (k/v projection duplicated across the 2 cores of a batch),
  - computes attention for all 16 heads over its query rows,
  - applies the output projection; host concatenates the 8 output chunks.
No collectives needed.

Layouts (all activations stored feature-major, "T" = [feature, seq]):
  qT/kT/vT inputs pre-transposed + bf16-cast on host.
  qhT [dout, qs], khT [dout, ks] from matmul(lhsT=W tile, rhs=xT tile).
  vh  [ks, dout] from matmul(lhsT=vT tile, rhs=Wv tile).
  scoresT [ks, qs] = matmul(lhsT=khT head slice, rhs=qhT head slice), two
    heads packed on the PE array via row tile_position (K=64 each).
  softmax: no max subtraction; a per-batch offset (host-computed from the
    mask; exact in fp32) keeps exponents bounded.  exp + mask-bias fused in
    one scalar-engine activation per psum group (bias is per-partition = per
    key position in the transposed layout).
  denominators: DVE accumulates sum over ks tiles, gpsimd partition_all_reduce
    sums over partitions; normalize after PV.
  PV: ctxT [dh, qs] = matmul(lhsT=vh slice, rhs=w tile), 2 heads packed via
    column tile_position.
  out: outT [do, qs] = matmul(lhsT=Wo tile, rhs=ctxT), host transposes.

Scale 1/sqrt(dk) folded into Wq on host. bq,bk folded into projection-eviction
biases; bv folded into bo (bo_eff = bo + bv @ Wo, exact since softmax rows
sum to 1).
"""

import os
import sys

for _p in ("/opt/trn_rl_repo", "/root/.axon_site/_ro/trn_rl_repo"):
    if os.path.isdir(_p) and _p not in sys.path:
        sys.path.insert(0, _p)

import numpy as np
import ml_dtypes

BF16 = ml_dtypes.bfloat16

P = 128
D = 1024
S = 2048
QS = 1024          # query rows per core
H = 16
DH = 64            # head depth
HP = 8             # head pairs
NDT = 8            # feature tiles (1024/128)
NKT = 16           # key tiles (2048/128)
NEG = np.float32(-1e10)

_CACHE = {}


def _build_program():
    import concourse.bass as bass
    import concourse.tile as tile
    from concourse import bacc, mybir, bass_isa

    f32 = mybir.dt.float32
    bf16 = mybir.dt.bfloat16
    ADD = mybir.AluOpType.add
    EXP = mybir.ActivationFunctionType.Exp

    nc = bacc.Bacc("TRN2", target_bir_lowering=False, debug=False)

    qT = nc.dram_tensor("qT", [D, QS], bf16, kind="ExternalInput").ap()
    kT = nc.dram_tensor("kT", [D, S], bf16, kind="ExternalInput").ap()
    vT = nc.dram_tensor("vT", [D, S], bf16, kind="ExternalInput").ap()
    wq = nc.dram_tensor("wq", [D, D], bf16, kind="ExternalInput").ap()
    wk = nc.dram_tensor("wk", [D, D], bf16, kind="ExternalInput").ap()
    wv = nc.dram_tensor("wv", [D, D], bf16, kind="ExternalInput").ap()
    wo = nc.dram_tensor("wo", [D, D], bf16, kind="ExternalInput").ap()
    mb = nc.dram_tensor("mb", [P, NKT], f32, kind="ExternalInput").ap()
    bqs = nc.dram_tensor("bqs", [P, NDT], f32, kind="ExternalInput").ap()
    bks = nc.dram_tensor("bks", [P, NDT], f32, kind="ExternalInput").ap()
    bos = nc.dram_tensor("bos", [P, NDT], f32, kind="ExternalInput").ap()
    outT = nc.dram_tensor("outT", [D, QS], f32, kind="ExternalOutput").ap()

    from contextlib import ExitStack

    with tile.TileContext(nc) as tc, ExitStack() as ctx:
        # ---- persistent SBUF ----
        per = ctx.enter_context(tc.tile_pool(name="persist", bufs=1))
        khT = per.tile([P, NDT * S], bf16, name="khT", tag="khT")     # 32KB/part
        qhT = per.tile([P, NDT * QS], bf16, name="qhT", tag="qhT")    # 16KB/part
        vh = per.tile([P, NKT * D], bf16, name="vh", tag="vh")       # 32KB/part
        ctxT = per.tile([P, HP * QS], bf16, name="ctxT", tag="ctxT")   # 16KB/part
        mb_sb = per.tile([P, NKT], f32, name="mb", tag="mb")
        bq_sb = per.tile([P, NDT], f32, name="bq", tag="bq")
        bk_sb = per.tile([P, NDT], f32, name="bk", tag="bk")
        bo_sb = per.tile([P, NDT], f32, name="bo", tag="bo")
        nc.sync.dma_start(out=mb_sb[:], in_=mb)
        nc.sync.dma_start(out=bq_sb[:], in_=bqs)
        nc.sync.dma_start(out=bk_sb[:], in_=bks)
        nc.sync.dma_start(out=bo_sb[:], in_=bos)

        # weights: 8 tiles per matrix, 32 tile allocations, 24 slots so Wo
        # streams in as the first-used weight frees.
        wts = ctx.enter_context(tc.tile_pool(name="wts", bufs=24))

        def load_w(w_dram):
            tiles = []
            for t in range(NDT):
                wt = wts.tile([P, D], bf16, name="w", tag="w")
                nc.sync.dma_start(out=wt[:], in_=w_dram[t * P:(t + 1) * P, :])
                tiles.append(wt)
            return tiles

        # ---- projections ----
        with tc.tile_pool(name="instream", bufs=8) as instream, \
             tc.tile_pool(name="proj_psum", bufs=2, space="PSUM") as proj_psum:

            # K projection: khT[dout, ks]
            wk_t = load_w(wk)
            kT_t = []
            for t in range(NDT):
                xt = instream.tile([P, S], bf16, name="xT", tag="xT")
                nc.sync.dma_start(out=xt[:], in_=kT[t * P:(t + 1) * P, :])
                kT_t.append(xt)
            for dt_ in range(NDT):
                for ck in range(4):
                    ps = proj_psum.tile([P, 512], f32, space="PSUM", name="pp", tag="pp")
                    for di in range(NDT):
                        nc.tensor.matmul(
                            ps[:],
                            lhsT=wk_t[di][:, dt_ * P:(dt_ + 1) * P],
                            rhs=kT_t[di][:, ck * 512:(ck + 1) * 512],
                            start=(di == 0), stop=(di == NDT - 1),
                        )
                    nc.vector.tensor_scalar(
                        out=khT[:, dt_ * S + ck * 512: dt_ * S + (ck + 1) * 512],
                        in0=ps[:], scalar1=bk_sb[:, dt_:dt_ + 1], scalar2=None,
                        op0=ADD,
                    )

            # Q projection: qhT[dout, qs] (scale already folded into wq/bq)
            wq_t = load_w(wq)
            qT_t = []
            for t in range(NDT):
                xt = instream.tile([P, S], bf16, name="xT", tag="xT")
                nc.sync.dma_start(out=xt[:, :QS], in_=qT[t * P:(t + 1) * P, :])
                qT_t.append(xt)
            for dt_ in range(NDT):
                for ck in range(2):
                    ps = proj_psum.tile([P, 512], f32, space="PSUM", name="pp", tag="pp")
                    for di in range(NDT):
                        nc.tensor.matmul(
                            ps[:],
                            lhsT=wq_t[di][:, dt_ * P:(dt_ + 1) * P],
                            rhs=qT_t[di][:, ck * 512:(ck + 1) * 512],
                            start=(di == 0), stop=(di == NDT - 1),
                        )
                    nc.vector.tensor_scalar(
                        out=qhT[:, dt_ * QS + ck * 512: dt_ * QS + (ck + 1) * 512],
                        in0=ps[:], scalar1=bq_sb[:, dt_:dt_ + 1], scalar2=None,
                        op0=ADD,
                    )

            # V projection: vh[ks, dout]
            wv_t = load_w(wv)
            vT_t = []
            for t in range(NDT):
                xt = instream.tile([P, S], bf16, name="xT", tag="xT")
                nc.sync.dma_start(out=xt[:], in_=vT[t * P:(t + 1) * P, :])
                vT_t.append(xt)
            for kt in range(NKT):
                for ck in range(2):
                    ps = proj_psum.tile([P, 512], f32, space="PSUM", name="pp", tag="pp")
                    for di in range(NDT):
                        nc.tensor.matmul(
                            ps[:],
                            lhsT=vT_t[di][:, kt * P:(kt + 1) * P],
                            rhs=wv_t[di][:, ck * 512:(ck + 1) * 512],
                            start=(di == 0), stop=(di == NDT - 1),
                        )
                    nc.vector.tensor_copy(
                        vh[:, kt * D + ck * 512: kt * D + (ck + 1) * 512],
                        ps[:],
                    )

        # ---- attention ----
        with tc.tile_pool(name="qk_psum", bufs=2, space="PSUM") as qk_psum, \
             tc.tile_pool(name="ctx_psum", bufs=4, space="PSUM") as ctx_psum, \
             tc.tile_pool(name="wprob", bufs=8) as wprob, \
             tc.tile_pool(name="sden", bufs=6) as sden:

            for hp in range(HP):
                kh_base = hp * S
                qh_base = hp * QS
                c0 = ctx_psum.tile([P, 512], f32, space="PSUM", name="ctxp", tag="ctxp")
                c1 = ctx_psum.tile([P, 512], f32, space="PSUM", name="ctxp", tag="ctxp")
                S_e = sden.tile([P, QS], f32, name="S", tag="S", bufs=4)
                S_o = sden.tile([P, QS], f32, name="S", tag="S", bufs=4)
                for kt in range(NKT):
                    ksl = slice(kh_base + kt * P, kh_base + (kt + 1) * P)
                    w_pair = []
                    for half, row0 in ((0, 0), (1, 64)):
                        qk = qk_psum.tile([P, QS], f32, space="PSUM", name="qk", tag="qk")
                        for ck in range(2):
                            nc.tensor.matmul(
                                qk[:, ck * 512:(ck + 1) * 512],
                                lhsT=khT[row0:row0 + DH, ksl],
                                rhs=qhT[row0:row0 + DH,
                                        qh_base + ck * 512: qh_base + (ck + 1) * 512],
                                start=True, stop=True,
                                tile_position=(row0, 0),
                            )
                        w = wprob.tile([P, QS], bf16, name="wprob", tag="wprob")
                        nc.scalar.activation(
                            w[:], qk[:], EXP,
                            bias=mb_sb[:, kt:kt + 1], scale=1.0,
                        )
                        w_pair.append(w)
                        S_t = S_e if half == 0 else S_o
                        if kt == 0:
                            nc.vector.tensor_copy(S_t[:], w[:])
                        else:
                            nc.vector.tensor_add(S_t[:], S_t[:], w[:])
                    w_e, w_o = w_pair
                    # PV, two heads packed by column groups
                    v_e = vh[:, kt * D + hp * P: kt * D + hp * P + DH]
                    v_o = vh[:, kt * D + hp * P + DH: kt * D + (hp + 1) * P]
                    for ck, cps in ((0, c0), (1, c1)):
                        csl = slice(ck * 512, (ck + 1) * 512)
                        nc.tensor.matmul(
                            cps[0:DH, :], lhsT=v_e, rhs=w_e[:, csl],
                            start=(kt == 0), stop=(kt == NKT - 1),
                            tile_position=(0, 0),
                        )
                        nc.tensor.matmul(
                            cps[DH:P, :], lhsT=v_o, rhs=w_o[:, csl],
                            start=(kt == 0), stop=(kt == NKT - 1),
                            tile_position=(0, 64),
                        )
                # denominators -> reciprocal, broadcast across partitions
                R_e = sden.tile([P, QS], f32, name="R", tag="R", bufs=4)
                R_o = sden.tile([P, QS], f32, name="R", tag="R", bufs=4)
                nc.gpsimd.partition_all_reduce(R_e[:], S_e[:], P,
                                               bass_isa.ReduceOp.add)
                nc.gpsimd.partition_all_reduce(R_o[:], S_o[:], P,
                                               bass_isa.ReduceOp.add)
                nc.vector.reciprocal(R_e[:], R_e[:])
                nc.vector.reciprocal(R_o[:], R_o[:])
                for ck, cps in ((0, c0), (1, c1)):
                    osl = slice(hp * QS + ck * 512, hp * QS + (ck + 1) * 512)
                    csl = slice(ck * 512, (ck + 1) * 512)
                    nc.vector.tensor_mul(
                        ctxT[0:DH, osl], cps[0:DH, :], R_e[0:DH, csl])
                    nc.vector.tensor_mul(
                        ctxT[DH:P, osl], cps[DH:P, :], R_o[DH:P, csl])

        # ---- output projection ----
        wo_t = load_w(wo)
        with tc.tile_pool(name="o_psum", bufs=2, space="PSUM") as o_psum, \
             tc.tile_pool(name="ostage", bufs=3) as ostage:
            for ck in range(2):
                for dt_ in range(NDT):
                    ps = o_psum.tile([P, 512], f32, space="PSUM", name="op", tag="op")
                    for hp in range(HP):
                        nc.tensor.matmul(
                            ps[:],
                            lhsT=wo_t[hp][:, dt_ * P:(dt_ + 1) * P],
                            rhs=ctxT[:, hp * QS + ck * 512: hp * QS + (ck + 1) * 512],
                            start=(hp == 0), stop=(hp == HP - 1),
                        )
                    o_sb = ostage.tile([P, 512], f32, name="o", tag="o")
                    nc.vector.tensor_scalar(
                        out=o_sb[:], in0=ps[:],
                        scalar1=bo_sb[:, dt_:dt_ + 1], scalar2=None, op0=ADD,
                    )
                    nc.sync.dma_start(
                        out=outT[dt_ * P:(dt_ + 1) * P, ck * 512:(ck + 1) * 512],
                        in_=o_sb[:],
                    )

    nc.compile()
    return nc


def _get_program():
    if "nc" not in _CACHE:
        _CACHE["nc"] = _build_program()
    return _CACHE["nc"]


def _prep_core_inputs(q, k, v, mask, Wq, bq, Wk, bk, Wv, bv, Wo, bo):
    """Host-side shard + transpose + cast. Returns list of 8 in_maps."""
    q = np.asarray(q, np.float32)
    k = np.asarray(k, np.float32)
    v = np.asarray(v, np.float32)
    mask = np.asarray(mask, np.float32)
    Wq = np.asarray(Wq, np.float32)
    Wk = np.asarray(Wk, np.float32)
    Wv = np.asarray(Wv, np.float32)
    Wo = np.asarray(Wo, np.float32)
    bq = np.asarray(bq, np.float32)
    bk = np.asarray(bk, np.float32)
    bv = np.asarray(bv, np.float32)
    bo = np.asarray(bo, np.float32)

    scale = np.float32(1.0 / np.sqrt(DH))
    wq_b = np.ascontiguousarray(Wq * scale).astype(BF16)
    wk_b = Wk.astype(BF16)
    wv_b = Wv.astype(BF16)
    wo_b = Wo.astype(BF16)
    bq_s = (bq * scale).astype(np.float32)
    bo_eff = (bo + bv @ Wo).astype(np.float32)

    def vec_tiles(x, ntiles):
        return np.ascontiguousarray(x.reshape(ntiles, P).T)  # [P, ntiles]

    in_maps = []
    for core in range(8):
        b, half = core // 2, core % 2
        mbv = mask[b, 0, 0] * NEG
        mbv = (mbv - mbv.max()).astype(np.float32)
        in_maps.append({
            "qT": np.ascontiguousarray(
                q[b, half * QS:(half + 1) * QS, :].T).astype(BF16),
            "kT": np.ascontiguousarray(k[b].T).astype(BF16),
            "vT": np.ascontiguousarray(v[b].T).astype(BF16),
            "wq": wq_b, "wk": wk_b, "wv": wv_b, "wo": wo_b,
            "mb": vec_tiles(mbv, NKT),
            "bqs": vec_tiles(bq_s, NDT),
            "bks": vec_tiles(bk, NDT),
            "bos": vec_tiles(bo_eff, NDT),
        })
    return in_maps


def kernel(q, k, v, mask, Wq, bq, Wk, bk, Wv, bv, Wo, bo):
    from concourse.bass_utils import run_bass_kernel_spmd

    nc = _get_program()
    in_maps = _prep_core_inputs(q, k, v, mask, Wq, bq, Wk, bk, Wv, bv, Wo, bo)
    res = run_bass_kernel_spmd(nc, in_maps, list(range(8)))
    B = q.shape[0]
    out = np.empty((B, S, D), np.float32)
    for core in range(8):
        b, half = core // 2, core % 2
        out[b, half * QS:(half + 1) * QS, :] = res.results[core]["outT"].T
    return out


if __name__ == "__main__":
    # smoke test with random data
    rng = np.random.default_rng(0)
    B = 4
    ins = dict(
        q=rng.standard_normal((B, S, D), np.float32),
        k=rng.standard_normal((B, S, D), np.float32),
        v=rng.standard_normal((B, S, D), np.float32),
        mask=rng.random((B, 1, 1, S), np.float32),
        Wq=rng.standard_normal((D, D), np.float32) / 32,
        bq=np.zeros(D, np.float32),
        Wk=rng.standard_normal((D, D), np.float32) / 32,
        bk=np.zeros(D, np.float32),
        Wv=rng.standard_normal((D, D), np.float32) / 32,
        bv=np.zeros(D, np.float32),
        Wo=rng.standard_normal((D, D), np.float32) / 32,
        bo=np.zeros(D, np.float32),
    )
    out = kernel(**ins)
    print("out", out.shape, out.dtype, np.abs(out).max())


# revision 6
# speedup vs baseline: 1.4914x; 1.4914x over previous
"""Trainium2 Bass kernel: MultiHeadAttention (B=4, S=2048, D=1024, H=16).

Sharding: 8 cores, each handles (batch b = core//2, query half = core%2):
projects q for its 1024 query rows, k/v for the full 2048-row sequence of its
batch, computes attention for all 16 heads, applies the output projection;
host concatenates the 8 output chunks. No collectives.

Layouts (feature-major activations, "T" = [feature, seq]):
  qhT [dout, qs], khT [dout, ks] from matmul(lhsT=W tile, rhs=xT tile).
  vh  [ks, dout] from matmul(lhsT=vT tile, rhs=Wv tile), stored augmented
    with a ones column per head ([ks, 65] blocks) so PV also produces the
    softmax denominator (row 64 of the PV psum).
  scoresT [ks, qs] via K=128 matmuls: khT stores head pairs (rows 0-63 even
    head, 64-127 odd head); qhT is stored zero-padded per head (the other
    64 rows are 0) so each head's QK matmul is a vanilla full-partition
    matmul (tile_position packing measured 2x slower than vanilla).
  softmax: no max subtraction; a per-batch offset (host-computed from the
    mask, exact fp32) keeps exponents bounded. exp + mask bias fused in one
    scalar-engine activation per [128,1024] psum group (bias per-partition =
    per key position in the transposed layout).
  normalize: denominator row -> gpsimd partition broadcast -> DVE
    reciprocal_approx_fast -> one DVE multiply per [64,512] ctx block.
  out: outT [do, qs] = matmul(lhsT=Wo tile, rhs=ctxT), host transposes.

Scale 1/sqrt(dk) folded into Wq on host. bq,bk folded into projection
eviction biases; bv folded into bo (bo_eff = bo + bv @ Wo, exact because
softmax rows sum to 1).
"""

import os
import sys

for _p in ("/opt/trn_rl_repo", "/root/.axon_site/_ro/trn_rl_repo"):
    if os.path.isdir(_p) and _p not in sys.path:
        sys.path.insert(0, _p)

import numpy as np
import ml_dtypes

BF16 = ml_dtypes.bfloat16

P = 128
D = 1024
S = 2048
QS = 1024          # query rows per core
H = 16
DH = 64            # head depth
DA = DH + 1        # augmented head width (ones column)
HP = 8             # head pairs
NDT = 8            # feature tiles (1024/128)
NKT = 16           # key tiles (2048/128)
NEG = np.float32(-1e10)

_CACHE = {}


def _build_program():
    import concourse.bass as bass
    import concourse.tile as tile
    from concourse import bacc, mybir

    f32 = mybir.dt.float32
    bf16 = mybir.dt.bfloat16
    ADD = mybir.AluOpType.add
    EXP = mybir.ActivationFunctionType.Exp

    nc = bacc.Bacc("TRN2", target_bir_lowering=False, debug=False)

    qT = nc.dram_tensor("qT", [D, QS], bf16, kind="ExternalInput").ap()
    kT = nc.dram_tensor("kT", [D, S], bf16, kind="ExternalInput").ap()
    vT = nc.dram_tensor("vT", [D, S], bf16, kind="ExternalInput").ap()
    wq = nc.dram_tensor("wq", [D, D], bf16, kind="ExternalInput").ap()
    wk = nc.dram_tensor("wk", [D, D], bf16, kind="ExternalInput").ap()
    wv = nc.dram_tensor("wv", [D, D], bf16, kind="ExternalInput").ap()
    wo = nc.dram_tensor("wo", [D, D], bf16, kind="ExternalInput").ap()
    mb = nc.dram_tensor("mb", [P, NKT], f32, kind="ExternalInput").ap()
    bqs = nc.dram_tensor("bqs", [P, NDT], f32, kind="ExternalInput").ap()
    bks = nc.dram_tensor("bks", [P, NDT], f32, kind="ExternalInput").ap()
    bos = nc.dram_tensor("bos", [P, NDT], f32, kind="ExternalInput").ap()
    outT = nc.dram_tensor("outT", [D, QS], f32, kind="ExternalOutput").ap()

    from contextlib import ExitStack

    with tile.TileContext(nc) as tc, ExitStack() as ctx:
        # ---- persistent SBUF ----
        per = ctx.enter_context(tc.tile_pool(name="persist", bufs=1))
        khT = per.tile([P, NDT * S], bf16, name="khT", tag="khT")        # 32KB
        qhp = per.tile([P, H * QS], bf16, name="qhp", tag="qhp")         # 32KB
        vha = per.tile([P, NKT * H * DA], bf16, name="vha", tag="vha")   # 32.5KB
        ctxT = per.tile([P, HP * QS], bf16, name="ctxT", tag="ctxT")     # 16KB
        mb_sb = per.tile([P, NKT], f32, name="mb", tag="mb")
        bq_sb = per.tile([P, NDT], f32, name="bq", tag="bq")
        bk_sb = per.tile([P, NDT], f32, name="bk", tag="bk")
        bo_sb = per.tile([P, NDT], f32, name="bo", tag="bo")
        nc.sync.dma_start(out=mb_sb[:], in_=mb)
        nc.sync.dma_start(out=bq_sb[:], in_=bqs)
        nc.sync.dma_start(out=bk_sb[:], in_=bks)
        nc.sync.dma_start(out=bo_sb[:], in_=bos)

        qhp3 = qhp.rearrange("p (h q) -> p h q", h=H)        # [128, 16, 1024]
        vha4 = vha.rearrange("p (t h e) -> p t h e", t=NKT, e=DA)

        # zero the unused half of each padded qh tile; ones columns of vha
        for h in range(H):
            if h % 2 == 0:
                nc.vector.memset(qhp3[DH:P, h, :], 0.0)
            else:
                nc.vector.memset(qhp3[0:DH, h, :], 0.0)
        for kt in range(NKT):
            nc.vector.memset(vha4[:, kt, :, DH:DA], 1.0)

        wts = ctx.enter_context(tc.tile_pool(name="wts", bufs=24))

        def load_w(w_dram):
            tiles = []
            for t in range(NDT):
                wt = wts.tile([P, D], bf16, name="w", tag="w")
                nc.sync.dma_start(out=wt[:], in_=w_dram[t * P:(t + 1) * P, :])
                tiles.append(wt)
            return tiles

        # ---- projections ----
        with tc.tile_pool(name="instream", bufs=8) as instream, \
             tc.tile_pool(name="proj_psum", bufs=2, space="PSUM") as proj_psum:

            # K projection: khT[dout, ks] (head pairs per 128-row tile)
            wk_t = load_w(wk)
            kT_t = []
            for t in range(NDT):
                xt = instream.tile([P, S], bf16, name="xT", tag="xT")
                nc.sync.dma_start(out=xt[:], in_=kT[t * P:(t + 1) * P, :])
                kT_t.append(xt)
            for dt_ in range(NDT):
                for ck in range(4):
                    ps = proj_psum.tile([P, 512], f32, space="PSUM",
                                        name="pp", tag="pp")
                    for di in range(NDT):
                        nc.tensor.matmul(
                            ps[:],
                            lhsT=wk_t[di][:, dt_ * P:(dt_ + 1) * P],
                            rhs=kT_t[di][:, ck * 512:(ck + 1) * 512],
                            start=(di == 0), stop=(di == NDT - 1),
                        )
                    nc.vector.tensor_scalar(
                        out=khT[:, dt_ * S + ck * 512: dt_ * S + (ck + 1) * 512],
                        in0=ps[:], scalar1=bk_sb[:, dt_:dt_ + 1], scalar2=None,
                        op0=ADD,
                    )

            # Q projection into zero-padded per-head tiles
            wq_t = load_w(wq)
            qT_t = []
            for t in range(NDT):
                xt = instream.tile([P, S], bf16, name="xT", tag="xT")
                nc.sync.dma_start(out=xt[:, :QS], in_=qT[t * P:(t + 1) * P, :])
                qT_t.append(xt)
            for dt_ in range(NDT):
                for ck in range(2):
                    ps = proj_psum.tile([P, 512], f32, space="PSUM",
                                        name="pp", tag="pp")
                    for di in range(NDT):
                        nc.tensor.matmul(
                            ps[:],
                            lhsT=wq_t[di][:, dt_ * P:(dt_ + 1) * P],
                            rhs=qT_t[di][:, ck * 512:(ck + 1) * 512],
                            start=(di == 0), stop=(di == NDT - 1),
                        )
                    csl = slice(ck * 512, (ck + 1) * 512)
                    nc.vector.tensor_scalar(
                        out=qhp3[0:DH, 2 * dt_, csl], in0=ps[0:DH, :],
                        scalar1=bq_sb[0:DH, dt_:dt_ + 1], scalar2=None, op0=ADD,
                    )
                    nc.vector.tensor_scalar(
                        out=qhp3[DH:P, 2 * dt_ + 1, csl], in0=ps[DH:P, :],
                        scalar1=bq_sb[DH:P, dt_:dt_ + 1], scalar2=None, op0=ADD,
                    )

            # V projection: vh[ks, dout] into augmented per-head blocks
            wv_t = load_w(wv)
            vT_t = []
            for t in range(NDT):
                xt = instream.tile([P, S], bf16, name="xT", tag="xT")
                nc.sync.dma_start(out=xt[:], in_=vT[t * P:(t + 1) * P, :])
                vT_t.append(xt)
            for kt in range(NKT):
                for ck in range(2):
                    ps = proj_psum.tile([P, 512], f32, space="PSUM",
                                        name="pp", tag="pp")
                    for di in range(NDT):
                        nc.tensor.matmul(
                            ps[:],
                            lhsT=vT_t[di][:, kt * P:(kt + 1) * P],
                            rhs=wv_t[di][:, ck * 512:(ck + 1) * 512],
                            start=(di == 0), stop=(di == NDT - 1),
                        )
                    nc.vector.tensor_copy(
                        vha4[:, kt, ck * 8:(ck + 1) * 8, 0:DH],
                        ps.rearrange("p (h d) -> p h d", d=DH),
                    )

        # ---- attention ----
        with tc.tile_pool(name="qk_psum", bufs=2, space="PSUM") as qk_psum, \
             tc.tile_pool(name="ctx_psum", bufs=4, space="PSUM") as ctx_psum, \
             tc.tile_pool(name="wprob", bufs=8) as wprob, \
             tc.tile_pool(name="norm", bufs=4) as norm:

            for h in range(H):
                hp = h // 2
                cps = [ctx_psum.tile([P, 512], f32, space="PSUM",
                                     name="ctxp", tag="ctxp")
                       for _ in range(2)]
                for kt in range(NKT):
                    qk = qk_psum.tile([P, QS], f32, space="PSUM",
                                      name="qk", tag="qk")
                    for ck in range(2):
                        nc.tensor.matmul(
                            qk[:, ck * 512:(ck + 1) * 512],
                            lhsT=khT[:, hp * S + kt * P: hp * S + (kt + 1) * P],
                            rhs=qhp3[:, h, ck * 512:(ck + 1) * 512],
                            start=True, stop=True,
                        )
                    w = wprob.tile([P, QS], bf16, name="wp", tag="wp")
                    nc.scalar.activation(
                        w[:], qk[:], EXP, bias=mb_sb[:, kt:kt + 1], scale=1.0,
                    )
                    for ck in range(2):
                        nc.tensor.matmul(
                            cps[ck][0:DA, :],
                            lhsT=vha4[:, kt, h, :],
                            rhs=w[:, ck * 512:(ck + 1) * 512],
                            start=(kt == 0), stop=(kt == NKT - 1),
                        )
                # normalize: denom row 64 -> broadcast -> recip -> multiply
                row0 = 0 if h % 2 == 0 else DH
                for ck in range(2):
                    den = norm.tile([1, 512], f32, name="den", tag="den")
                    nc.vector.tensor_copy(den[:], cps[ck][DH:DA, :])
                    rb = norm.tile([DH, 512], f32, name="rb", tag="rb")
                    nc.gpsimd.partition_broadcast(rb[:], den[0:1, :])
                    rc = norm.tile([DH, 512], f32, name="rc", tag="rc")
                    nc.vector.reciprocal_approx_fast(out=rc[:], in_=rb[:])
                    osl = slice(hp * QS + ck * 512, hp * QS + (ck + 1) * 512)
                    nc.vector.tensor_mul(
                        ctxT[row0:row0 + DH, osl], cps[ck][0:DH, :], rc[:])

        # ---- output projection ----
        wo_t = load_w(wo)
        with tc.tile_pool(name="o_psum", bufs=2, space="PSUM") as o_psum, \
             tc.tile_pool(name="ostage", bufs=3) as ostage:
            for ck in range(2):
                for dt_ in range(NDT):
                    ps = o_psum.tile([P, 512], f32, space="PSUM",
                                     name="op", tag="op")
                    for hp in range(HP):
                        nc.tensor.matmul(
                            ps[:],
                            lhsT=wo_t[hp][:, dt_ * P:(dt_ + 1) * P],
                            rhs=ctxT[:, hp * QS + ck * 512: hp * QS + (ck + 1) * 512],
                            start=(hp == 0), stop=(hp == HP - 1),
                        )
                    o_sb = ostage.tile([P, 512], f32, name="o", tag="o")
                    nc.vector.tensor_scalar(
                        out=o_sb[:], in0=ps[:],
                        scalar1=bo_sb[:, dt_:dt_ + 1], scalar2=None, op0=ADD,
                    )
                    nc.sync.dma_start(
                        out=outT[dt_ * P:(dt_ + 1) * P, ck * 512:(ck + 1) * 512],
                        in_=o_sb[:],
                    )

    nc.compile()
    return nc


def _get_program():
    if "nc" not in _CACHE:
        _CACHE["nc"] = _build_program()
    return _CACHE["nc"]


def _prep_core_inputs(q, k, v, mask, Wq, bq, Wk, bk, Wv, bv, Wo, bo):
    """Host-side shard + transpose + cast. Returns list of 8 in_maps."""
    q = np.asarray(q, np.float32)
    k = np.asarray(k, np.float32)
    v = np.asarray(v, np.float32)
    mask = np.asarray(mask, np.float32)
    Wq = np.asarray(Wq, np.float32)
    Wk = np.asarray(Wk, np.float32)
    Wv = np.asarray(Wv, np.float32)
    Wo = np.asarray(Wo, np.float32)
    bq = np.asarray(bq, np.float32)
    bk = np.asarray(bk, np.float32)
    bv = np.asarray(bv, np.float32)
    bo = np.asarray(bo, np.float32)

    scale = np.float32(1.0 / np.sqrt(DH))
    wq_b = np.ascontiguousarray(Wq * scale).astype(BF16)
    wk_b = Wk.astype(BF16)
    wv_b = Wv.astype(BF16)
    wo_b = Wo.astype(BF16)
    bq_s = (bq * scale).astype(np.float32)
    bo_eff = (bo + bv @ Wo).astype(np.float32)

    def vec_tiles(x, ntiles):
        return np.ascontiguousarray(x.reshape(ntiles, P).T)  # [P, ntiles]

    in_maps = []
    for core in range(8):
        b, half = core // 2, core % 2
        mbv = mask[b, 0, 0] * NEG
        mbv = (mbv - mbv.max()).astype(np.float32)
        in_maps.append({
            "qT": np.ascontiguousarray(
                q[b, half * QS:(half + 1) * QS, :].T).astype(BF16),
            "kT": np.ascontiguousarray(k[b].T).astype(BF16),
            "vT": np.ascontiguousarray(v[b].T).astype(BF16),
            "wq": wq_b, "wk": wk_b, "wv": wv_b, "wo": wo_b,
            "mb": vec_tiles(mbv, NKT),
            "bqs": vec_tiles(bq_s, NDT),
            "bks": vec_tiles(bk, NDT),
            "bos": vec_tiles(bo_eff, NDT),
        })
    return in_maps


def kernel(q, k, v, mask, Wq, bq, Wk, bk, Wv, bv, Wo, bo):
    from concourse.bass_utils import run_bass_kernel_spmd

    nc = _get_program()
    in_maps = _prep_core_inputs(q, k, v, mask, Wq, bq, Wk, bk, Wv, bv, Wo, bo)
    res = run_bass_kernel_spmd(nc, in_maps, list(range(8)))
    B = q.shape[0]
    out = np.empty((B, S, D), np.float32)
    for core in range(8):
        b, half = core // 2, core % 2
        out[b, half * QS:(half + 1) * QS, :] = res.results[core]["outT"].T
    return out
